# revision 35
# baseline (speedup 1.0000x reference)
"""Trainium2 Bass kernel for a binarized-conv BasicBlock (dense_cnn).

Computation (matches the reference nn.Module):
    out = clip(BN2(conv3x3(binarize(clip(BN1(conv3x3(binarize(x), binarize(w1))))),
                  binarize(w2)) + x))
with training-mode (batch-stats) BN over the full 64-image batch.

Strategy:
  - Data-parallel over batch: 8 images per core on 8 NeuronCores.
  - Binarized 3x3 conv as 18 accumulating PE matmuls per output tile
    (9 taps x 2 input-channel blocks of 128) over zero-padded [128, 30x30]
    activation tiles; +-1 values in bf16 are exact, accumulation is fp32 PSUM.
  - BN1 + hardtanh + binarize collapses to a per-channel threshold compare
    (hardtanh does not change the sign); binarize is exactly
    is_ge(y1, thresh) - 0.5 (times 2, folded into BN2's affine), which also
    matches binarize(0) == +1 at the boundary.
  - Sync-BN: per-channel sum / sum-of-squares partials are AllReduce'd
    across the 8 cores ([128, 4] fp32 = 2KB, twice).
  - conv2 inputs are +-0.5 (is_ge output minus 0.5); the residual add is a
    single fused DVE scalar_tensor_tensor: z = (psum * 2) + x with the
    per-channel sum accumulated in the same instruction.
"""

import os
import sys

import numpy as np


def _ensure_paths():
    for p in ("/opt/trn_rl_repo", "/root/.axon_site/_ro/trn_rl_repo"):
        if p not in sys.path and os.path.isdir(p):
            sys.path.append(p)


try:
    from concourse import bacc, mybir, tile  # noqa: F401
except ImportError:
    _ensure_paths()
    from concourse import bacc, mybir, tile  # noqa: F401

from concourse.bass_utils import run_bass_kernel_spmd
from concourse.masks import make_identity

N_CORES = 8
IMGS = 8          # images per core (64 / 8)
C = 256
CB = 2            # channel blocks of 128
H = W = 28
HP = WP = 30      # zero-padded spatial
PIX = H * W       # 784
HALF = PIX // 2   # 392 (one PSUM bank of fp32)
NT = 64 * PIX     # BN count over the GLOBAL batch (N*H*W)
EPS = 1e-5

F32 = mybir.dt.float32
BF16 = mybir.dt.bfloat16
FP8 = mybir.dt.float8e4
AF = mybir.ActivationFunctionType
ALU = mybir.AluOpType
DR = mybir.MatmulPerfMode.DoubleRow

# padded fp8 activation layout: [128, 2 kblocks, 30 rows, 32 cols]
RP = 32           # row pitch (28 cols + pad, %16 bytes)
KP = HP * RP      # per-kblock pitch = 960

_PROGRAM = None


def _build_program():
    nc = bacc.Bacc("TRN2", target_bir_lowering=False, debug=False,
                   num_devices=N_CORES)

    x_in = nc.dram_tensor("x", [IMGS, C, H, W], F32, kind="ExternalInput").ap()
    w1_in = nc.dram_tensor("w1", [C, C, 3, 3], F32, kind="ExternalInput").ap()
    w2_in = nc.dram_tensor("w2", [C, C, 3, 3], F32, kind="ExternalInput").ap()
    g1_in = nc.dram_tensor("gamma1", [C], F32, kind="ExternalInput").ap()
    b1_in = nc.dram_tensor("beta1", [C], F32, kind="ExternalInput").ap()
    g2_in = nc.dram_tensor("gamma2", [C], F32, kind="ExternalInput").ap()
    b2_in = nc.dram_tensor("beta2", [C], F32, kind="ExternalInput").ap()
    out_d = nc.dram_tensor("out", [IMGS, C, H, W], F32, kind="ExternalOutput").ap()

    groups = [list(range(N_CORES))]

    with tile.TileContext(nc) as tc:
        with (
            tc.tile_pool(name="consts", bufs=1) as p_const,
            tc.tile_pool(name="wstage", bufs=2) as p_wstage,
            tc.tile_pool(name="wt", bufs=2 * 9 * 2) as p_wt,
            tc.tile_pool(name="xp", bufs=IMGS * CB) as p_x,
            tc.tile_pool(name="apad", bufs=IMGS + 2) as p_apad,
            tc.tile_pool(name="yz", bufs=IMGS * CB) as p_yz,
            tc.tile_pool(name="sq", bufs=2) as p_sq,
            tc.tile_pool(name="o1", bufs=6) as p_o1,
            tc.tile_pool(name="ps", bufs=8, space="PSUM") as p_ps,
            tc.tile_pool(name="dram", bufs=1, space="DRAM") as p_dram,
        ):
            # Warmup collective: absorbs the ~11us ncfw wake + first-mesh-op
            # overhead on stream 0 while conv1 runs, so the BN sync
            # AllReduces behave like warm ops (~10us instead of ~38us).
            ccw_i = p_dram.tile([128, 1], F32, name="ccw_i")
            ccw_o = p_dram.tile([128, 1], F32, name="ccw_o")
            zz = p_const.tile([128, 1], F32, name="zz")
            nc.vector.memset(zz, 0.0)
            nc.sync.dma_start(out=ccw_i, in_=zz)
            nc.gpsimd.collective_compute(
                "AllReduce", ALU.add, replica_groups=groups,
                ins=[ccw_i.opt()], outs=[ccw_o.opt()])

            ident = p_const.tile([128, 128], F32, name="ident")
            make_identity(nc, ident)

            def dma_chunked(out_ap, in_ap, parts, alt=True):
                """Split a big DMA along the last free dim so the transfer
                spreads across several DMA engines (~23 GB/s each).  Chunk
                issues alternate between the sync (HWDGE) and gpsimd (SWDGE)
                sequencers so the ~0.7us per-issue cost is parallelized."""
                n = out_ap.shape[-1]
                step = (n + parts - 1) // parts
                for ci, a in enumerate(range(0, n, step)):
                    b = min(a + step, n)
                    eng = nc.gpsimd if (alt and ci % 2) else nc.sync
                    eng.dma_start(out=out_ap[:, a:b], in_=in_ap[:, a:b])

            # gamma/beta as [128, 2] (col = channel block)
            def load_cvec(src, nm):
                t = p_const.tile([128, CB], F32, name=nm)
                nc.sync.dma_start(out=t, in_=src.rearrange("(b p) -> p b", p=128))
                return t

            g1t = load_cvec(g1_in, "g1t")
            b1t = load_cvec(b1_in, "b1t")
            g2t = load_cvec(g2_in, "g2t")
            b2t = load_cvec(b2_in, "b2t")

            # per-channel stat accumulators, one column per (img, half)
            def stat_tiles(nm):
                return [p_const.tile([128, IMGS * 2], F32, name=f"{nm}{ob}")
                        for ob in range(CB)]

            st1s, st1q = stat_tiles("st1s"), stat_tiles("st1q")
            st2s, st2q = stat_tiles("st2s"), stat_tiles("st2q")

            # ---- weight prep: sign(w)^T as DoubleRow fp8 [128 i, 2 kb, 128 o] ----
            def prep_weights(w_in, wi, obs=None, wt=None):
                wt = {} if wt is None else wt
                for ob in (range(CB) if obs is None else obs):
                    wst = p_wstage.tile([128, C * 9], F32, tag="wst",
                                        name=f"wst{wi}_{ob}")
                    dma_chunked(
                        wst,
                        w_in[ob * 128:(ob + 1) * 128].rearrange(
                            "o i ky kx -> o (i ky kx)"),
                        parts=4)
                    w3 = wst.rearrange("p (i t) -> p i t", t=9)
                    for tap in range(9):
                        t = p_wt.tile([128, CB * 128], FP8, tag="wt",
                                      name=f"wt{wi}_{tap}_{ob}")
                        wt[(tap, ob)] = t
                        for kb in range(CB):
                            ps = p_ps.tile([128, 128], F32, tag="ps",
                                           name=f"pst{wi}_{ob}_{kb}_{tap}")
                            nc.tensor.transpose(
                                ps, w3[:, kb * 128:(kb + 1) * 128, tap], ident)
                            nc.scalar.activation(
                                t[:, kb * 128:(kb + 1) * 128], ps, AF.Sign)
                return wt

            # ---- x: load raw f32 (kept for the residual), sign -> padded fp8 ----
            xsign = [None] * IMGS
            xt = [[None] * CB for _ in range(IMGS)]

            def load_x(n, parts=2):
                ap = p_apad.tile([128, CB * KP], FP8, tag="apad",
                                 name=f"xs_{n}")
                nc.gpsimd.memset(ap, 0.0)
                xsign[n] = ap
                a4 = ap.rearrange("p (k r c) -> p k r c", k=CB, r=HP)
                for b in range(CB):
                    xr = p_x.tile([128, PIX], F32, tag="xp", name=f"x_{n}_{b}")
                    eng = nc.gpsimd if b else nc.sync
                    eng.dma_start(
                        out=xr,
                        in_=x_in[n, b * 128:(b + 1) * 128].rearrange(
                            "c h w -> c (h w)"))
                    xt[n][b] = xr
                    nc.scalar.activation(
                        out=a4[:, b, 1:29, 1:29],
                        in_=xr.rearrange("p (h w) -> p h w", h=H),
                        func=AF.Sign)

            # order the head so conv1 can start early: w1[ob0] staging DMA
            # leads the queue, then the first image pair, then w1[ob1].
            for n in (0, 1):
                load_x(n)
            wt1 = prep_weights(w1_in, 1, obs=[0])
            for n in (2, 3):
                load_x(n)
            prep_weights(w1_in, 1, obs=[1], wt=wt1)
            for n in range(4, IMGS):
                load_x(n)

            # ---- conv: 9 DoubleRow matmuls (K=256) per [128, 392] PSUM tile ----
            def do_conv(wt, act, evict):
                for pair in range(IMGS // 2):
                    for ob in range(CB):
                        tiles = [(n, half)
                                 for n in (2 * pair, 2 * pair + 1)
                                 for half in range(2)]
                        pss = {}
                        for (n, half) in tiles:
                            pss[(n, half)] = p_ps.tile(
                                [128, HALF], F32, tag="ps",
                                name=f"ps_{ob}_{n}_{half}")
                        for tap in range(9):
                            dy, dx = divmod(tap, 3)
                            w3 = wt[(tap, ob)].rearrange(
                                "p (k o) -> p k o", k=CB)
                            for (n, half) in tiles:
                                a4 = act[n].rearrange(
                                    "p (k r c) -> p k r c", k=CB, r=HP)
                                rhs = a4[:, :, dy + half * 14: dy + half * 14 + 14,
                                         dx: dx + W]
                                nc.tensor.matmul(pss[(n, half)], w3, rhs,
                                                 start=(tap == 0),
                                                 stop=(tap == 8),
                                                 perf_mode=DR)
                        for (n, half) in tiles:
                            evict(n, ob, half, pss[(n, half)])

            # ---- conv1 eviction: copy PSUM->y1 with sum, square with sumsq ----
            y1 = [[None] * CB for _ in range(IMGS)]

            def evict1(n, ob, half, ps):
                if y1[n][ob] is None:
                    y1[n][ob] = p_yz.tile([128, PIX], F32, tag="yz",
                                          name=f"y1_{n}_{ob}")
                idx = n * 2 + half
                ysl = y1[n][ob][:, half * HALF:(half + 1) * HALF]
                nc.scalar.activation(ysl, ps, AF.Copy,
                                     accum_out=st1s[ob][:, idx:idx + 1])
                sq = p_sq.tile([128, HALF], F32, tag="sq")
                nc.vector.scalar_tensor_tensor(
                    out=sq, in0=ysl, scalar=1.0, in1=ysl,
                    op0=ALU.mult, op1=ALU.mult,
                    accum_out=st1q[ob][:, idx:idx + 1])

            do_conv(wt1, xsign, evict1)

            # ---- BN1: AllReduce global sums, derive per-channel thresholds ----
            pk1 = p_const.tile([128, 2 * CB], F32, name="pk1")
            for ob in range(CB):
                nc.vector.tensor_reduce(out=pk1[:, 2 * ob:2 * ob + 1],
                                        in_=st1s[ob], axis=mybir.AxisListType.X,
                                        op=ALU.add)
                nc.vector.tensor_reduce(out=pk1[:, 2 * ob + 1:2 * ob + 2],
                                        in_=st1q[ob], axis=mybir.AxisListType.X,
                                        op=ALU.add)
            cc1i = p_dram.tile([128, 2 * CB], F32, name="cc1i")
            cc1o = p_dram.tile([128, 2 * CB], F32, name="cc1o")
            nc.sync.dma_start(out=cc1i, in_=pk1)
            nc.gpsimd.collective_compute(
                "AllReduce", ALU.add, replica_groups=groups,
                ins=[cc1i.opt()], outs=[cc1o.opt()])
            red1 = p_const.tile([128, 2 * CB], F32, name="red1")
            nc.sync.dma_start(out=red1, in_=cc1o)
            r3 = red1.rearrange("p (b k) -> p b k", k=2)

            # w2 prep + b2a memsets fill the sync-BN wait on PE/GpSimd
            wt2 = prep_weights(w2_in, 2)
            b2a = [None] * IMGS
            for n in range(IMGS):
                ap = p_apad.tile([128, CB * KP], FP8, tag="apad",
                                 name=f"b2_{n}")
                nc.gpsimd.memset(ap, 0.0)
                b2a[n] = ap

            m1 = p_const.tile([128, CB], F32, name="m1")
            nc.vector.tensor_scalar(out=m1, in0=r3[:, :, 0], scalar1=1.0 / NT,
                                    scalar2=None, op0=ALU.mult)
            e1 = p_const.tile([128, CB], F32, name="e1")
            nc.vector.tensor_scalar(out=e1, in0=r3[:, :, 1], scalar1=1.0 / NT,
                                    scalar2=None, op0=ALU.mult)
            mm1 = p_const.tile([128, CB], F32, name="mm1")
            nc.vector.tensor_mul(mm1, m1, m1)
            v1 = p_const.tile([128, CB], F32, name="v1")
            nc.vector.tensor_sub(v1, e1, mm1)
            v1e = p_const.tile([128, CB], F32, name="v1e")
            nc.vector.tensor_scalar(out=v1e, in0=v1, scalar1=EPS, scalar2=None,
                                    op0=ALU.add)
            rc1 = p_const.tile([128, CB], F32, name="rc1")
            nc.vector.reciprocal(rc1, v1e)
            rstd1 = p_const.tile([128, CB], F32, name="rstd1")
            nc.scalar.activation(rstd1, rc1, AF.Sqrt)
            sc1 = p_const.tile([128, CB], F32, name="sc1")
            nc.vector.tensor_mul(sc1, g1t, rstd1)
            rsc1 = p_const.tile([128, CB], F32, name="rsc1")
            nc.vector.reciprocal(rsc1, sc1)
            tb1 = p_const.tile([128, CB], F32, name="tb1")
            nc.vector.tensor_mul(tb1, b1t, rsc1)
            thr1 = p_const.tile([128, CB], F32, name="thr1")
            nc.vector.tensor_sub(thr1, m1, tb1)

            # ---- binarize(BN1(y1)) == is_ge(y1, thr) - 0.5 (padded fp8) ----
            for n in range(IMGS):
                a4 = b2a[n].rearrange("p (k r c) -> p k r c", k=CB, r=HP)
                for b in range(CB):
                    nc.vector.tensor_scalar(
                        out=a4[:, b, 1:29, 1:29],
                        in0=y1[n][b].rearrange("p (h w) -> p h w", h=H),
                        scalar1=thr1[:, b:b + 1], scalar2=0.5,
                        op0=ALU.is_ge, op1=ALU.subtract)

            # ---- conv2 eviction: z = 2*psum + x (fused sum), square ----
            z = [[None] * CB for _ in range(IMGS)]

            def evict2(n, ob, half, ps):
                if z[n][ob] is None:
                    z[n][ob] = p_yz.tile([128, PIX], F32, tag="yz",
                                         name=f"z_{n}_{ob}")
                idx = n * 2 + half
                zsl = z[n][ob][:, half * HALF:(half + 1) * HALF]
                nc.vector.scalar_tensor_tensor(
                    out=zsl, in0=ps, scalar=2.0,
                    in1=xt[n][ob][:, half * HALF:(half + 1) * HALF],
                    op0=ALU.mult, op1=ALU.add,
                    accum_out=st2s[ob][:, idx:idx + 1])
                sq = p_sq.tile([128, HALF], F32, tag="sq")
                nc.scalar.activation(sq, zsl, AF.Square,
                                     accum_out=st2q[ob][:, idx:idx + 1])

            do_conv(wt2, b2a, evict2)

            # ---- BN2 on z (true values): fscale = gamma2*rstd2 ----
            pk2 = p_const.tile([128, 2 * CB], F32, name="pk2")
            for ob in range(CB):
                nc.vector.tensor_reduce(out=pk2[:, 2 * ob:2 * ob + 1],
                                        in_=st2s[ob], axis=mybir.AxisListType.X,
                                        op=ALU.add)
                nc.vector.tensor_reduce(out=pk2[:, 2 * ob + 1:2 * ob + 2],
                                        in_=st2q[ob], axis=mybir.AxisListType.X,
                                        op=ALU.add)
            cc2i = p_dram.tile([128, 2 * CB], F32, name="cc2i")
            cc2o = p_dram.tile([128, 2 * CB], F32, name="cc2o")
            nc.sync.dma_start(out=cc2i, in_=pk2)
            nc.gpsimd.collective_compute(
                "AllReduce", ALU.add, replica_groups=groups,
                ins=[cc2i.opt()], outs=[cc2o.opt()])
            red2 = p_const.tile([128, 2 * CB], F32, name="red2")
            nc.sync.dma_start(out=red2, in_=cc2o)
            q3 = red2.rearrange("p (b k) -> p b k", k=2)

            m2 = p_const.tile([128, CB], F32, name="m2")
            nc.vector.tensor_scalar(out=m2, in0=q3[:, :, 0], scalar1=1.0 / NT,
                                    scalar2=None, op0=ALU.mult)
            e2 = p_const.tile([128, CB], F32, name="e2")
            nc.vector.tensor_scalar(out=e2, in0=q3[:, :, 1], scalar1=1.0 / NT,
                                    scalar2=None, op0=ALU.mult)
            mm2 = p_const.tile([128, CB], F32, name="mm2")
            nc.vector.tensor_mul(mm2, m2, m2)
            v2 = p_const.tile([128, CB], F32, name="v2")
            nc.vector.tensor_sub(v2, e2, mm2)
            v2f = p_const.tile([128, CB], F32, name="v2f")
            nc.vector.tensor_scalar(out=v2f, in0=v2, scalar1=EPS, scalar2=None,
                                    op0=ALU.add)
            rc2 = p_const.tile([128, CB], F32, name="rc2")
            nc.vector.reciprocal(rc2, v2f)
            rstd2 = p_const.tile([128, CB], F32, name="rstd2")
            nc.scalar.activation(rstd2, rc2, AF.Sqrt)
            fscale = p_const.tile([128, CB], F32, name="fscale")
            nc.vector.tensor_mul(fscale, g2t, rstd2)
            msc = p_const.tile([128, CB], F32, name="msc")
            nc.vector.tensor_mul(msc, m2, fscale)
            fbias = p_const.tile([128, CB], F32, name="fbias")
            nc.vector.tensor_sub(fbias, b2t, msc)

            # ---- final: clip(z * fscale + fbias) -> DRAM ----
            # affine split across ScalarE (ob=0) / VectorE (ob=1); clamp on DVE
            for n in range(IMGS):
                for ob in range(CB):
                    o1 = p_o1.tile([128, PIX], F32, tag="o1")
                    if ob == 0:
                        nc.scalar.activation(o1, z[n][ob], AF.Identity,
                                             bias=fbias[:, ob:ob + 1],
                                             scale=fscale[:, ob:ob + 1])
                    else:
                        nc.vector.tensor_scalar(
                            out=o1, in0=z[n][ob],
                            scalar1=fscale[:, ob:ob + 1],
                            scalar2=fbias[:, ob:ob + 1],
                            op0=ALU.mult, op1=ALU.add)
                    ceng = nc.gpsimd if ob == 0 else nc.vector
                    ceng.tensor_scalar(out=o1, in0=o1, scalar1=-1.0,
                                       scalar2=1.0, op0=ALU.max,
                                       op1=ALU.min)
                    dma_chunked(
                        out_d[n, ob * 128:(ob + 1) * 128].rearrange(
                            "c h w -> c (h w)"),
                        o1, parts=2)

    nc.compile()
    return nc


def _get_program():
    global _PROGRAM
    if _PROGRAM is None:
        _PROGRAM = _build_program()
    return _PROGRAM


def run_sharded(inputs, **spmd_kwargs):
    """Shard inputs across 8 cores, run, and gather. Returns (out, results)."""
    nc = _get_program()
    x = np.ascontiguousarray(np.asarray(inputs["x"], dtype=np.float32))
    base = {
        k: np.ascontiguousarray(np.asarray(inputs[k], dtype=np.float32))
        for k in ("w1", "w2", "gamma1", "beta1", "gamma2", "beta2")
    }
    shards = np.split(x, N_CORES, axis=0)
    in_maps = [{"x": shards[i], **base} for i in range(N_CORES)]
    res = run_bass_kernel_spmd(nc, in_maps, core_ids=list(range(N_CORES)),
                               **spmd_kwargs)
    out = np.concatenate([res.results[i]["out"] for i in range(N_CORES)],
                         axis=0).astype(np.float32)
    return out, res


def kernel(**inputs):
    out, _ = run_sharded(inputs)
    return out


# revision 36
# speedup vs baseline: 1.3999x; 1.3999x over previous
"""Trainium2 Bass kernel for a binarized-conv BasicBlock (dense_cnn).

Computation (matches the reference nn.Module):
    out = clip(BN2(conv3x3(binarize(clip(BN1(conv3x3(binarize(x), binarize(w1))))),
                  binarize(w2)) + x))
with training-mode (batch-stats) BN over the full 64-image batch.

Strategy:
  - Data-parallel over batch: 8 images per core on 8 NeuronCores.
  - Binarized 3x3 conv as 18 accumulating PE matmuls per output tile
    (9 taps x 2 input-channel blocks of 128) over zero-padded [128, 30x30]
    activation tiles; +-1 values in bf16 are exact, accumulation is fp32 PSUM.
  - BN1 + hardtanh + binarize collapses to a per-channel threshold compare
    (hardtanh does not change the sign); binarize is exactly
    is_ge(y1, thresh) - 0.5 (times 2, folded into BN2's affine), which also
    matches binarize(0) == +1 at the boundary.
  - Sync-BN: per-channel sum / sum-of-squares partials are AllReduce'd
    across the 8 cores ([128, 4] fp32 = 2KB, twice).
  - conv2 inputs are +-0.5 (is_ge output minus 0.5); the residual add is a
    single fused DVE scalar_tensor_tensor: z = (psum * 2) + x with the
    per-channel sum accumulated in the same instruction.
"""

import os
import sys

import numpy as np


def _ensure_paths():
    for p in ("/opt/trn_rl_repo", "/root/.axon_site/_ro/trn_rl_repo"):
        if p not in sys.path and os.path.isdir(p):
            sys.path.append(p)


try:
    from concourse import bacc, mybir, tile  # noqa: F401
except ImportError:
    _ensure_paths()
    from concourse import bacc, mybir, tile  # noqa: F401

from concourse.bass_utils import run_bass_kernel_spmd
from concourse.masks import make_identity

N_CORES = 8
IMGS = 8          # images per core (64 / 8)
C = 256
CB = 2            # channel blocks of 128
H = W = 28
HP = WP = 30      # zero-padded spatial
PIX = H * W       # 784
HALF = PIX // 2   # 392 (one PSUM bank of fp32)
NT = 64 * PIX     # BN count over the GLOBAL batch (N*H*W)
EPS = 1e-5

F32 = mybir.dt.float32
BF16 = mybir.dt.bfloat16
FP8 = mybir.dt.float8e4
AF = mybir.ActivationFunctionType
ALU = mybir.AluOpType
DR = mybir.MatmulPerfMode.DoubleRow

# padded fp8 activation layout: [128, 2 kblocks, 30 rows, 32 cols]
RP = 32           # row pitch (28 cols + pad, %16 bytes)
KP = HP * RP      # per-kblock pitch = 960

_PROGRAM = None


def _build_program():
    nc = bacc.Bacc("TRN2", target_bir_lowering=False, debug=False,
                   num_devices=N_CORES)

    x_in = nc.dram_tensor("x", [IMGS, C, H, W], F32, kind="ExternalInput").ap()
    w1_in = nc.dram_tensor("w1", [C, C, 3, 3], F32, kind="ExternalInput").ap()
    w2_in = nc.dram_tensor("w2", [C, C, 3, 3], F32, kind="ExternalInput").ap()
    g1_in = nc.dram_tensor("gamma1", [C], F32, kind="ExternalInput").ap()
    b1_in = nc.dram_tensor("beta1", [C], F32, kind="ExternalInput").ap()
    g2_in = nc.dram_tensor("gamma2", [C], F32, kind="ExternalInput").ap()
    b2_in = nc.dram_tensor("beta2", [C], F32, kind="ExternalInput").ap()
    out_d = nc.dram_tensor("out", [IMGS, C, H, W], F32, kind="ExternalOutput").ap()

    groups = [list(range(N_CORES))]

    with tile.TileContext(nc) as tc:
        with (
            tc.tile_pool(name="consts", bufs=1) as p_const,
            tc.tile_pool(name="wstage", bufs=2) as p_wstage,
            tc.tile_pool(name="wt", bufs=2 * 9 * 2) as p_wt,
            tc.tile_pool(name="xp", bufs=IMGS * CB) as p_x,
            tc.tile_pool(name="apad", bufs=IMGS + 2) as p_apad,
            tc.tile_pool(name="yz", bufs=IMGS * CB) as p_yz,
            tc.tile_pool(name="sq", bufs=2) as p_sq,
            tc.tile_pool(name="o1", bufs=6) as p_o1,
            tc.tile_pool(name="ps", bufs=8, space="PSUM") as p_ps,
            tc.tile_pool(name="dram", bufs=1, space="DRAM") as p_dram,
        ):
            # Warmup collective: absorbs the ~11us ncfw wake + first-mesh-op
            # overhead on stream 0 while conv1 runs, so the BN sync
            # AllReduces behave like warm ops (~10us instead of ~38us).
            ccw_i = p_dram.tile([128, 1], F32, name="ccw_i")
            ccw_o = p_dram.tile([128, 1], F32, name="ccw_o")
            zz = p_const.tile([128, 1], F32, name="zz")
            nc.vector.memset(zz, 0.0)
            nc.sync.dma_start(out=ccw_i, in_=zz)
            nc.gpsimd.collective_compute(
                "AllReduce", ALU.add, replica_groups=groups,
                ins=[ccw_i.opt()], outs=[ccw_o.opt()])

            ident = p_const.tile([128, 128], F32, name="ident")
            make_identity(nc, ident)

            def dma_chunked(out_ap, in_ap, parts, alt=True):
                """Split a big DMA along the last free dim so the transfer
                spreads across several DMA engines (~23 GB/s each).  Chunk
                issues alternate between the sync (HWDGE) and gpsimd (SWDGE)
                sequencers so the ~0.7us per-issue cost is parallelized."""
                n = out_ap.shape[-1]
                step = (n + parts - 1) // parts
                for ci, a in enumerate(range(0, n, step)):
                    b = min(a + step, n)
                    eng = nc.gpsimd if (alt and ci % 2) else nc.sync
                    eng.dma_start(out=out_ap[:, a:b], in_=in_ap[:, a:b])

            # gamma/beta as [128, 2] (col = channel block)
            def load_cvec(src, nm):
                t = p_const.tile([128, CB], F32, name=nm)
                nc.sync.dma_start(out=t, in_=src.rearrange("(b p) -> p b", p=128))
                return t

            g1t = load_cvec(g1_in, "g1t")
            b1t = load_cvec(b1_in, "b1t")
            g2t = load_cvec(g2_in, "g2t")
            b2t = load_cvec(b2_in, "b2t")

            # per-channel stat accumulators, one column per (img, half)
            def stat_tiles(nm):
                return [p_const.tile([128, IMGS * 2], F32, name=f"{nm}{ob}")
                        for ob in range(CB)]

            st1s, st1q = stat_tiles("st1s"), stat_tiles("st1q")
            st2s, st2q = stat_tiles("st2s"), stat_tiles("st2q")

            # ---- weight prep: sign(w)^T as DoubleRow fp8 [128 i, 2 kb, 128 o] ----
            def prep_weights(w_in, wi, obs=None, wt=None):
                wt = {} if wt is None else wt
                for ob in (range(CB) if obs is None else obs):
                    wst = p_wstage.tile([128, C * 9], F32, tag="wst",
                                        name=f"wst{wi}_{ob}")
                    nc.sync.dma_start(
                        out=wst,
                        in_=w_in[ob * 128:(ob + 1) * 128].rearrange(
                            "o i ky kx -> o (i ky kx)"))
                    w3 = wst.rearrange("p (i t) -> p i t", t=9)
                    for tap in range(9):
                        t = p_wt.tile([128, CB * 128], FP8, tag="wt",
                                      name=f"wt{wi}_{tap}_{ob}")
                        wt[(tap, ob)] = t
                        for kb in range(CB):
                            ps = p_ps.tile([128, 128], F32, tag="ps",
                                           name=f"pst{wi}_{ob}_{kb}_{tap}")
                            nc.tensor.transpose(
                                ps, w3[:, kb * 128:(kb + 1) * 128, tap], ident)
                            nc.scalar.activation(
                                t[:, kb * 128:(kb + 1) * 128], ps, AF.Sign)
                return wt

            # ---- x: load raw f32 (kept for the residual), sign -> padded fp8 ----
            xsign = [None] * IMGS
            xt = [[None] * CB for _ in range(IMGS)]

            def load_x(n, parts=2):
                ap = p_apad.tile([128, CB * KP], FP8, tag="apad",
                                 name=f"xs_{n}")
                nc.gpsimd.memset(ap, 0.0)
                xsign[n] = ap
                a4 = ap.rearrange("p (k r c) -> p k r c", k=CB, r=HP)
                for b in range(CB):
                    xr = p_x.tile([128, PIX], F32, tag="xp", name=f"x_{n}_{b}")
                    nc.sync.dma_start(
                        out=xr,
                        in_=x_in[n, b * 128:(b + 1) * 128].rearrange(
                            "c h w -> c (h w)"))
                    xt[n][b] = xr
                    nc.scalar.activation(
                        out=a4[:, b, 1:29, 1:29],
                        in_=xr.rearrange("p (h w) -> p h w", h=H),
                        func=AF.Sign)

            # order the head so conv1 can start early: w1[ob0] staging DMA
            # leads the queue, then the first image pair, then w1[ob1].
            wt1 = prep_weights(w1_in, 1, obs=[0])
            for n in (0, 1):
                load_x(n)
            prep_weights(w1_in, 1, obs=[1], wt=wt1)
            for n in range(2, IMGS):
                load_x(n)

            # ---- conv: 9 DoubleRow matmuls (K=256) per [128, 392] PSUM tile ----
            def do_conv(wt, act, evict):
                for pair in range(IMGS // 2):
                    for ob in range(CB):
                        tiles = [(n, half)
                                 for n in (2 * pair, 2 * pair + 1)
                                 for half in range(2)]
                        pss = {}
                        for (n, half) in tiles:
                            pss[(n, half)] = p_ps.tile(
                                [128, HALF], F32, tag="ps",
                                name=f"ps_{ob}_{n}_{half}")
                        for tap in range(9):
                            dy, dx = divmod(tap, 3)
                            w3 = wt[(tap, ob)].rearrange(
                                "p (k o) -> p k o", k=CB)
                            for (n, half) in tiles:
                                a4 = act[n].rearrange(
                                    "p (k r c) -> p k r c", k=CB, r=HP)
                                rhs = a4[:, :, dy + half * 14: dy + half * 14 + 14,
                                         dx: dx + W]
                                nc.tensor.matmul(pss[(n, half)], w3, rhs,
                                                 start=(tap == 0),
                                                 stop=(tap == 8),
                                                 perf_mode=DR)
                        for (n, half) in tiles:
                            evict(n, ob, half, pss[(n, half)])

            # ---- conv1 eviction: copy PSUM->y1 with sum, square with sumsq ----
            y1 = [[None] * CB for _ in range(IMGS)]

            def evict1(n, ob, half, ps):
                if y1[n][ob] is None:
                    y1[n][ob] = p_yz.tile([128, PIX], F32, tag="yz",
                                          name=f"y1_{n}_{ob}")
                idx = n * 2 + half
                ysl = y1[n][ob][:, half * HALF:(half + 1) * HALF]
                nc.scalar.activation(ysl, ps, AF.Copy,
                                     accum_out=st1s[ob][:, idx:idx + 1])
                sq = p_sq.tile([128, HALF], F32, tag="sq")
                nc.vector.scalar_tensor_tensor(
                    out=sq, in0=ysl, scalar=1.0, in1=ysl,
                    op0=ALU.mult, op1=ALU.mult,
                    accum_out=st1q[ob][:, idx:idx + 1])

            do_conv(wt1, xsign, evict1)

            # ---- BN1: AllReduce global sums, derive per-channel thresholds ----
            pk1 = p_const.tile([128, 2 * CB], F32, name="pk1")
            for ob in range(CB):
                nc.vector.tensor_reduce(out=pk1[:, 2 * ob:2 * ob + 1],
                                        in_=st1s[ob], axis=mybir.AxisListType.X,
                                        op=ALU.add)
                nc.vector.tensor_reduce(out=pk1[:, 2 * ob + 1:2 * ob + 2],
                                        in_=st1q[ob], axis=mybir.AxisListType.X,
                                        op=ALU.add)
            cc1i = p_dram.tile([128, 2 * CB], F32, name="cc1i")
            cc1o = p_dram.tile([128, 2 * CB], F32, name="cc1o")
            nc.sync.dma_start(out=cc1i, in_=pk1)
            nc.gpsimd.collective_compute(
                "AllReduce", ALU.add, replica_groups=groups,
                ins=[cc1i.opt()], outs=[cc1o.opt()])
            red1 = p_const.tile([128, 2 * CB], F32, name="red1")
            nc.sync.dma_start(out=red1, in_=cc1o)
            r3 = red1.rearrange("p (b k) -> p b k", k=2)

            # w2 prep + b2a memsets fill the sync-BN wait on PE/GpSimd
            wt2 = prep_weights(w2_in, 2)
            b2a = [None] * IMGS
            for n in range(IMGS):
                ap = p_apad.tile([128, CB * KP], FP8, tag="apad",
                                 name=f"b2_{n}")
                nc.gpsimd.memset(ap, 0.0)
                b2a[n] = ap

            m1 = p_const.tile([128, CB], F32, name="m1")
            nc.vector.tensor_scalar(out=m1, in0=r3[:, :, 0], scalar1=1.0 / NT,
                                    scalar2=None, op0=ALU.mult)
            e1 = p_const.tile([128, CB], F32, name="e1")
            nc.vector.tensor_scalar(out=e1, in0=r3[:, :, 1], scalar1=1.0 / NT,
                                    scalar2=None, op0=ALU.mult)
            mm1 = p_const.tile([128, CB], F32, name="mm1")
            nc.vector.tensor_mul(mm1, m1, m1)
            v1 = p_const.tile([128, CB], F32, name="v1")
            nc.vector.tensor_sub(v1, e1, mm1)
            v1e = p_const.tile([128, CB], F32, name="v1e")
            nc.vector.tensor_scalar(out=v1e, in0=v1, scalar1=EPS, scalar2=None,
                                    op0=ALU.add)
            rc1 = p_const.tile([128, CB], F32, name="rc1")
            nc.vector.reciprocal(rc1, v1e)
            rstd1 = p_const.tile([128, CB], F32, name="rstd1")
            nc.scalar.activation(rstd1, rc1, AF.Sqrt)
            sc1 = p_const.tile([128, CB], F32, name="sc1")
            nc.vector.tensor_mul(sc1, g1t, rstd1)
            rsc1 = p_const.tile([128, CB], F32, name="rsc1")
            nc.vector.reciprocal(rsc1, sc1)
            tb1 = p_const.tile([128, CB], F32, name="tb1")
            nc.vector.tensor_mul(tb1, b1t, rsc1)
            thr1 = p_const.tile([128, CB], F32, name="thr1")
            nc.vector.tensor_sub(thr1, m1, tb1)

            # ---- binarize(BN1(y1)) == is_ge(y1, thr) - 0.5 (padded fp8) ----
            for n in range(IMGS):
                a4 = b2a[n].rearrange("p (k r c) -> p k r c", k=CB, r=HP)
                for b in range(CB):
                    nc.vector.tensor_scalar(
                        out=a4[:, b, 1:29, 1:29],
                        in0=y1[n][b].rearrange("p (h w) -> p h w", h=H),
                        scalar1=thr1[:, b:b + 1], scalar2=0.5,
                        op0=ALU.is_ge, op1=ALU.subtract)

            # ---- conv2 eviction: z = 2*psum + x (fused sum), square ----
            z = [[None] * CB for _ in range(IMGS)]

            def evict2(n, ob, half, ps):
                if z[n][ob] is None:
                    z[n][ob] = p_yz.tile([128, PIX], F32, tag="yz",
                                         name=f"z_{n}_{ob}")
                idx = n * 2 + half
                zsl = z[n][ob][:, half * HALF:(half + 1) * HALF]
                nc.vector.scalar_tensor_tensor(
                    out=zsl, in0=ps, scalar=2.0,
                    in1=xt[n][ob][:, half * HALF:(half + 1) * HALF],
                    op0=ALU.mult, op1=ALU.add,
                    accum_out=st2s[ob][:, idx:idx + 1])
                sq = p_sq.tile([128, HALF], F32, tag="sq")
                nc.scalar.activation(sq, zsl, AF.Square,
                                     accum_out=st2q[ob][:, idx:idx + 1])

            do_conv(wt2, b2a, evict2)

            # ---- BN2 on z (true values): fscale = gamma2*rstd2 ----
            pk2 = p_const.tile([128, 2 * CB], F32, name="pk2")
            for ob in range(CB):
                nc.vector.tensor_reduce(out=pk2[:, 2 * ob:2 * ob + 1],
                                        in_=st2s[ob], axis=mybir.AxisListType.X,
                                        op=ALU.add)
                nc.vector.tensor_reduce(out=pk2[:, 2 * ob + 1:2 * ob + 2],
                                        in_=st2q[ob], axis=mybir.AxisListType.X,
                                        op=ALU.add)
            cc2i = p_dram.tile([128, 2 * CB], F32, name="cc2i")
            cc2o = p_dram.tile([128, 2 * CB], F32, name="cc2o")
            nc.sync.dma_start(out=cc2i, in_=pk2)
            nc.gpsimd.collective_compute(
                "AllReduce", ALU.add, replica_groups=groups,
                ins=[cc2i.opt()], outs=[cc2o.opt()])
            red2 = p_const.tile([128, 2 * CB], F32, name="red2")
            nc.sync.dma_start(out=red2, in_=cc2o)
            q3 = red2.rearrange("p (b k) -> p b k", k=2)

            m2 = p_const.tile([128, CB], F32, name="m2")
            nc.vector.tensor_scalar(out=m2, in0=q3[:, :, 0], scalar1=1.0 / NT,
                                    scalar2=None, op0=ALU.mult)
            e2 = p_const.tile([128, CB], F32, name="e2")
            nc.vector.tensor_scalar(out=e2, in0=q3[:, :, 1], scalar1=1.0 / NT,
                                    scalar2=None, op0=ALU.mult)
            mm2 = p_const.tile([128, CB], F32, name="mm2")
            nc.vector.tensor_mul(mm2, m2, m2)
            v2 = p_const.tile([128, CB], F32, name="v2")
            nc.vector.tensor_sub(v2, e2, mm2)
            v2f = p_const.tile([128, CB], F32, name="v2f")
            nc.vector.tensor_scalar(out=v2f, in0=v2, scalar1=EPS, scalar2=None,
                                    op0=ALU.add)
            rc2 = p_const.tile([128, CB], F32, name="rc2")
            nc.vector.reciprocal(rc2, v2f)
            rstd2 = p_const.tile([128, CB], F32, name="rstd2")
            nc.scalar.activation(rstd2, rc2, AF.Sqrt)
            fscale = p_const.tile([128, CB], F32, name="fscale")
            nc.vector.tensor_mul(fscale, g2t, rstd2)
            msc = p_const.tile([128, CB], F32, name="msc")
            nc.vector.tensor_mul(msc, m2, fscale)
            fbias = p_const.tile([128, CB], F32, name="fbias")
            nc.vector.tensor_sub(fbias, b2t, msc)

            # ---- final: clip(z * fscale + fbias) -> DRAM ----
            # affine split across ScalarE (ob=0) / VectorE (ob=1); clamp on DVE
            for n in range(IMGS):
                for ob in range(CB):
                    o1 = p_o1.tile([128, PIX], F32, tag="o1")
                    if ob == 0:
                        nc.scalar.activation(o1, z[n][ob], AF.Identity,
                                             bias=fbias[:, ob:ob + 1],
                                             scale=fscale[:, ob:ob + 1])
                    else:
                        nc.vector.tensor_scalar(
                            out=o1, in0=z[n][ob],
                            scalar1=fscale[:, ob:ob + 1],
                            scalar2=fbias[:, ob:ob + 1],
                            op0=ALU.mult, op1=ALU.add)
                    nc.vector.tensor_scalar(out=o1, in0=o1, scalar1=-1.0,
                                            scalar2=1.0, op0=ALU.max,
                                            op1=ALU.min)
                    dma_chunked(
                        out_d[n, ob * 128:(ob + 1) * 128].rearrange(
                            "c h w -> c (h w)"),
                        o1, parts=2)

    nc.compile()
    return nc


def _get_program():
    global _PROGRAM
    if _PROGRAM is None:
        _PROGRAM = _build_program()
    return _PROGRAM


def run_sharded(inputs, **spmd_kwargs):
    """Shard inputs across 8 cores, run, and gather. Returns (out, results)."""
    nc = _get_program()
    x = np.ascontiguousarray(np.asarray(inputs["x"], dtype=np.float32))
    base = {
        k: np.ascontiguousarray(np.asarray(inputs[k], dtype=np.float32))
        for k in ("w1", "w2", "gamma1", "beta1", "gamma2", "beta2")
    }
    shards = np.split(x, N_CORES, axis=0)
    in_maps = [{"x": shards[i], **base} for i in range(N_CORES)]
    res = run_bass_kernel_spmd(nc, in_maps, core_ids=list(range(N_CORES)),
                               **spmd_kwargs)
    out = np.concatenate([res.results[i]["out"] for i in range(N_CORES)],
                         axis=0).astype(np.float32)
    return out, res


def kernel(**inputs):
    out, _ = run_sharded(inputs)
    return out


# revision 37
# speedup vs baseline: 1.4918x; 1.0657x over previous
"""Trainium2 Bass kernel for a binarized-conv BasicBlock (dense_cnn).

Computation (matches the reference nn.Module):
    out = clip(BN2(conv3x3(binarize(clip(BN1(conv3x3(binarize(x), binarize(w1))))),
                  binarize(w2)) + x))
with training-mode (batch-stats) BN over the full 64-image batch.

Strategy:
  - Data-parallel over batch: 8 images per core on 8 NeuronCores.
  - Binarized 3x3 conv as 18 accumulating PE matmuls per output tile
    (9 taps x 2 input-channel blocks of 128) over zero-padded [128, 30x30]
    activation tiles; +-1 values in bf16 are exact, accumulation is fp32 PSUM.
  - BN1 + hardtanh + binarize collapses to a per-channel threshold compare
    (hardtanh does not change the sign); binarize is exactly
    is_ge(y1, thresh) - 0.5 (times 2, folded into BN2's affine), which also
    matches binarize(0) == +1 at the boundary.
  - Sync-BN: per-channel sum / sum-of-squares partials are AllReduce'd
    across the 8 cores ([128, 4] fp32 = 2KB, twice).
  - conv2 inputs are +-0.5 (is_ge output minus 0.5); the residual add is a
    single fused DVE scalar_tensor_tensor: z = (psum * 2) + x with the
    per-channel sum accumulated in the same instruction.
"""

import os
import sys

import numpy as np


def _ensure_paths():
    for p in ("/opt/trn_rl_repo", "/root/.axon_site/_ro/trn_rl_repo"):
        if p not in sys.path and os.path.isdir(p):
            sys.path.append(p)


try:
    from concourse import bacc, mybir, tile  # noqa: F401
except ImportError:
    _ensure_paths()
    from concourse import bacc, mybir, tile  # noqa: F401

from concourse.bass_utils import run_bass_kernel_spmd
from concourse.masks import make_identity

N_CORES = 8
IMGS = 8          # images per core (64 / 8)
C = 256
CB = 2            # channel blocks of 128
H = W = 28
HP = WP = 30      # zero-padded spatial
PIX = H * W       # 784
HALF = PIX // 2   # 392 (one PSUM bank of fp32)
NT = 64 * PIX     # BN count over the GLOBAL batch (N*H*W)
EPS = 1e-5

F32 = mybir.dt.float32
BF16 = mybir.dt.bfloat16
FP8 = mybir.dt.float8e4
AF = mybir.ActivationFunctionType
ALU = mybir.AluOpType
DR = mybir.MatmulPerfMode.DoubleRow

# padded fp8 activation layout: [128, 2 kblocks, 30 rows, 32 cols]
RP = 32           # row pitch (28 cols + pad, %16 bytes)
KP = HP * RP      # per-kblock pitch = 960

_PROGRAM = None


def _build_program():
    nc = bacc.Bacc("TRN2", target_bir_lowering=False, debug=False,
                   num_devices=N_CORES)

    x_in = nc.dram_tensor("x", [IMGS, C, H, W], F32, kind="ExternalInput").ap()
    w1_in = nc.dram_tensor("w1", [C, C, 3, 3], F32, kind="ExternalInput").ap()
    w2_in = nc.dram_tensor("w2", [C, C, 3, 3], F32, kind="ExternalInput").ap()
    g1_in = nc.dram_tensor("gamma1", [C], F32, kind="ExternalInput").ap()
    b1_in = nc.dram_tensor("beta1", [C], F32, kind="ExternalInput").ap()
    g2_in = nc.dram_tensor("gamma2", [C], F32, kind="ExternalInput").ap()
    b2_in = nc.dram_tensor("beta2", [C], F32, kind="ExternalInput").ap()
    out_d = nc.dram_tensor("out", [IMGS, C, H, W], F32, kind="ExternalOutput").ap()

    groups = [list(range(N_CORES))]

    with tile.TileContext(nc) as tc:
        with (
            tc.tile_pool(name="consts", bufs=1) as p_const,
            tc.tile_pool(name="wstage", bufs=2) as p_wstage,
            tc.tile_pool(name="wt", bufs=2 * 9 * 2) as p_wt,
            tc.tile_pool(name="xp", bufs=IMGS * CB) as p_x,
            tc.tile_pool(name="apad", bufs=IMGS + 2) as p_apad,
            tc.tile_pool(name="yz", bufs=IMGS * CB) as p_yz,
            tc.tile_pool(name="sq", bufs=2) as p_sq,
            tc.tile_pool(name="o1", bufs=6) as p_o1,
            tc.tile_pool(name="ps", bufs=8, space="PSUM") as p_ps,
            tc.tile_pool(name="dram", bufs=1, space="DRAM") as p_dram,
        ):
            # Warmup collective: absorbs the ~11us ncfw wake + first-mesh-op
            # overhead on stream 0 while conv1 runs, so the BN sync
            # AllReduces behave like warm ops (~10us instead of ~38us).
            ccw_i = p_dram.tile([128, 1], F32, name="ccw_i")
            ccw_o = p_dram.tile([128, 1], F32, name="ccw_o")
            zz = p_const.tile([128, 1], F32, name="zz")
            nc.vector.memset(zz, 0.0)
            nc.sync.dma_start(out=ccw_i, in_=zz)
            nc.gpsimd.collective_compute(
                "AllReduce", ALU.add, replica_groups=groups,
                ins=[ccw_i.opt()], outs=[ccw_o.opt()])

            ident = p_const.tile([128, 128], F32, name="ident")
            make_identity(nc, ident)

            def dma_chunked(out_ap, in_ap, parts, alt=True):
                """Split a big DMA along the last free dim so the transfer
                spreads across several DMA engines (~23 GB/s each).  Chunk
                issues alternate between the sync (HWDGE) and gpsimd (SWDGE)
                sequencers so the ~0.7us per-issue cost is parallelized."""
                n = out_ap.shape[-1]
                step = (n + parts - 1) // parts
                for ci, a in enumerate(range(0, n, step)):
                    b = min(a + step, n)
                    eng = nc.gpsimd if (alt and ci % 2) else nc.sync
                    eng.dma_start(out=out_ap[:, a:b], in_=in_ap[:, a:b])

            # gamma/beta as [128, 2] (col = channel block)
            def load_cvec(src, nm):
                t = p_const.tile([128, CB], F32, name=nm)
                nc.sync.dma_start(out=t, in_=src.rearrange("(b p) -> p b", p=128))
                return t

            g1t = load_cvec(g1_in, "g1t")
            b1t = load_cvec(b1_in, "b1t")
            g2t = load_cvec(g2_in, "g2t")
            b2t = load_cvec(b2_in, "b2t")

            # per-channel stat accumulators, one column per (img, half)
            def stat_tiles(nm):
                return [p_const.tile([128, IMGS * 2], F32, name=f"{nm}{ob}")
                        for ob in range(CB)]

            st1s, st1q = stat_tiles("st1s"), stat_tiles("st1q")
            st2s, st2q = stat_tiles("st2s"), stat_tiles("st2q")

            # ---- weight prep: sign(w)^T as DoubleRow fp8 [128 i, 2 kb, 128 o] ----
            def prep_weights(w_in, wi, obs=None, wt=None):
                wt = {} if wt is None else wt
                for ob in (range(CB) if obs is None else obs):
                    wst = p_wstage.tile([128, C * 9], F32, tag="wst",
                                        name=f"wst{wi}_{ob}")
                    dma_chunked(
                        wst,
                        w_in[ob * 128:(ob + 1) * 128].rearrange(
                            "o i ky kx -> o (i ky kx)"),
                        parts=3, alt=False)
                    w3 = wst.rearrange("p (i t) -> p i t", t=9)
                    for tap in range(9):
                        t = p_wt.tile([128, CB * 128], FP8, tag="wt",
                                      name=f"wt{wi}_{tap}_{ob}")
                        wt[(tap, ob)] = t
                        for kb in range(CB):
                            ps = p_ps.tile([128, 128], F32, tag="ps",
                                           name=f"pst{wi}_{ob}_{kb}_{tap}")
                            nc.tensor.transpose(
                                ps, w3[:, kb * 128:(kb + 1) * 128, tap], ident)
                            nc.scalar.activation(
                                t[:, kb * 128:(kb + 1) * 128], ps, AF.Sign)
                return wt

            # ---- x: load raw f32 (kept for the residual), sign -> padded fp8 ----
            xsign = [None] * IMGS
            xt = [[None] * CB for _ in range(IMGS)]

            def load_x(n, parts=1):
                ap = p_apad.tile([128, CB * KP], FP8, tag="apad",
                                 name=f"xs_{n}")
                nc.gpsimd.memset(ap, 0.0)
                xsign[n] = ap
                a4 = ap.rearrange("p (k r c) -> p k r c", k=CB, r=HP)
                for b in range(CB):
                    xr = p_x.tile([128, PIX], F32, tag="xp", name=f"x_{n}_{b}")
                    if parts > 1:
                        dma_chunked(
                            xr,
                            x_in[n, b * 128:(b + 1) * 128].rearrange(
                                "c h w -> c (h w)"),
                            parts=parts, alt=False)
                    else:
                        nc.sync.dma_start(
                            out=xr,
                            in_=x_in[n, b * 128:(b + 1) * 128].rearrange(
                                "c h w -> c (h w)"))
                    xt[n][b] = xr
                    nc.scalar.activation(
                        out=a4[:, b, 1:29, 1:29],
                        in_=xr.rearrange("p (h w) -> p h w", h=H),
                        func=AF.Sign)

            # order the head so conv1 can start early: w1[ob0] staging DMA
            # leads the queue, then the first image pair, then w1[ob1].
            wt1 = prep_weights(w1_in, 1, obs=[0])
            for n in (0, 1):
                load_x(n, parts=2)
            prep_weights(w1_in, 1, obs=[1], wt=wt1)
            for n in range(2, IMGS):
                load_x(n)

            # ---- conv: 9 DoubleRow matmuls (K=256) per [128, 392] PSUM tile ----
            def do_conv(wt, act, evict):
                for pair in range(IMGS // 2):
                    for ob in range(CB):
                        tiles = [(n, half)
                                 for n in (2 * pair, 2 * pair + 1)
                                 for half in range(2)]
                        pss = {}
                        for (n, half) in tiles:
                            pss[(n, half)] = p_ps.tile(
                                [128, HALF], F32, tag="ps",
                                name=f"ps_{ob}_{n}_{half}")
                        for tap in range(9):
                            dy, dx = divmod(tap, 3)
                            w3 = wt[(tap, ob)].rearrange(
                                "p (k o) -> p k o", k=CB)
                            for (n, half) in tiles:
                                a4 = act[n].rearrange(
                                    "p (k r c) -> p k r c", k=CB, r=HP)
                                rhs = a4[:, :, dy + half * 14: dy + half * 14 + 14,
                                         dx: dx + W]
                                nc.tensor.matmul(pss[(n, half)], w3, rhs,
                                                 start=(tap == 0),
                                                 stop=(tap == 8),
                                                 perf_mode=DR)
                        for (n, half) in tiles:
                            evict(n, ob, half, pss[(n, half)])

            # ---- conv1 eviction: copy PSUM->y1 with sum, square with sumsq ----
            y1 = [[None] * CB for _ in range(IMGS)]

            def evict1(n, ob, half, ps):
                if y1[n][ob] is None:
                    y1[n][ob] = p_yz.tile([128, PIX], F32, tag="yz",
                                          name=f"y1_{n}_{ob}")
                idx = n * 2 + half
                ysl = y1[n][ob][:, half * HALF:(half + 1) * HALF]
                nc.scalar.activation(ysl, ps, AF.Copy,
                                     accum_out=st1s[ob][:, idx:idx + 1])
                sq = p_sq.tile([128, HALF], F32, tag="sq")
                nc.vector.scalar_tensor_tensor(
                    out=sq, in0=ysl, scalar=1.0, in1=ysl,
                    op0=ALU.mult, op1=ALU.mult,
                    accum_out=st1q[ob][:, idx:idx + 1])

            do_conv(wt1, xsign, evict1)

            # ---- BN1: AllReduce global sums, derive per-channel thresholds ----
            pk1 = p_const.tile([128, 2 * CB], F32, name="pk1")
            for ob in range(CB):
                nc.vector.tensor_reduce(out=pk1[:, 2 * ob:2 * ob + 1],
                                        in_=st1s[ob], axis=mybir.AxisListType.X,
                                        op=ALU.add)
                nc.vector.tensor_reduce(out=pk1[:, 2 * ob + 1:2 * ob + 2],
                                        in_=st1q[ob], axis=mybir.AxisListType.X,
                                        op=ALU.add)
            cc1i = p_dram.tile([128, 2 * CB], F32, name="cc1i")
            cc1o = p_dram.tile([128, 2 * CB], F32, name="cc1o")
            nc.sync.dma_start(out=cc1i, in_=pk1)
            nc.gpsimd.collective_compute(
                "AllReduce", ALU.add, replica_groups=groups,
                ins=[cc1i.opt()], outs=[cc1o.opt()])
            red1 = p_const.tile([128, 2 * CB], F32, name="red1")
            nc.sync.dma_start(out=red1, in_=cc1o)
            r3 = red1.rearrange("p (b k) -> p b k", k=2)

            # w2 prep + b2a memsets fill the sync-BN wait on PE/GpSimd
            wt2 = prep_weights(w2_in, 2)
            b2a = [None] * IMGS
            for n in range(IMGS):
                ap = p_apad.tile([128, CB * KP], FP8, tag="apad",
                                 name=f"b2_{n}")
                nc.gpsimd.memset(ap, 0.0)
                b2a[n] = ap

            m1 = p_const.tile([128, CB], F32, name="m1")
            nc.vector.tensor_scalar(out=m1, in0=r3[:, :, 0], scalar1=1.0 / NT,
                                    scalar2=None, op0=ALU.mult)
            e1 = p_const.tile([128, CB], F32, name="e1")
            nc.vector.tensor_scalar(out=e1, in0=r3[:, :, 1], scalar1=1.0 / NT,
                                    scalar2=None, op0=ALU.mult)
            mm1 = p_const.tile([128, CB], F32, name="mm1")
            nc.vector.tensor_mul(mm1, m1, m1)
            v1 = p_const.tile([128, CB], F32, name="v1")
            nc.vector.tensor_sub(v1, e1, mm1)
            v1e = p_const.tile([128, CB], F32, name="v1e")
            nc.vector.tensor_scalar(out=v1e, in0=v1, scalar1=EPS, scalar2=None,
                                    op0=ALU.add)
            rc1 = p_const.tile([128, CB], F32, name="rc1")
            nc.vector.reciprocal(rc1, v1e)
            rstd1 = p_const.tile([128, CB], F32, name="rstd1")
            nc.scalar.activation(rstd1, rc1, AF.Sqrt)
            sc1 = p_const.tile([128, CB], F32, name="sc1")
            nc.vector.tensor_mul(sc1, g1t, rstd1)
            rsc1 = p_const.tile([128, CB], F32, name="rsc1")
            nc.vector.reciprocal(rsc1, sc1)
            tb1 = p_const.tile([128, CB], F32, name="tb1")
            nc.vector.tensor_mul(tb1, b1t, rsc1)
            thr1 = p_const.tile([128, CB], F32, name="thr1")
            nc.vector.tensor_sub(thr1, m1, tb1)

            # ---- binarize(BN1(y1)) == is_ge(y1, thr) - 0.5 (padded fp8) ----
            for n in range(IMGS):
                a4 = b2a[n].rearrange("p (k r c) -> p k r c", k=CB, r=HP)
                for b in range(CB):
                    nc.vector.tensor_scalar(
                        out=a4[:, b, 1:29, 1:29],
                        in0=y1[n][b].rearrange("p (h w) -> p h w", h=H),
                        scalar1=thr1[:, b:b + 1], scalar2=0.5,
                        op0=ALU.is_ge, op1=ALU.subtract)

            # ---- conv2 eviction: z = 2*psum + x (fused sum), square ----
            z = [[None] * CB for _ in range(IMGS)]

            def evict2(n, ob, half, ps):
                if z[n][ob] is None:
                    z[n][ob] = p_yz.tile([128, PIX], F32, tag="yz",
                                         name=f"z_{n}_{ob}")
                idx = n * 2 + half
                zsl = z[n][ob][:, half * HALF:(half + 1) * HALF]
                nc.vector.scalar_tensor_tensor(
                    out=zsl, in0=ps, scalar=2.0,
                    in1=xt[n][ob][:, half * HALF:(half + 1) * HALF],
                    op0=ALU.mult, op1=ALU.add,
                    accum_out=st2s[ob][:, idx:idx + 1])
                sq = p_sq.tile([128, HALF], F32, tag="sq")
                nc.scalar.activation(sq, zsl, AF.Square,
                                     accum_out=st2q[ob][:, idx:idx + 1])

            do_conv(wt2, b2a, evict2)

            # ---- BN2 on z (true values): fscale = gamma2*rstd2 ----
            pk2 = p_const.tile([128, 2 * CB], F32, name="pk2")
            for ob in range(CB):
                nc.vector.tensor_reduce(out=pk2[:, 2 * ob:2 * ob + 1],
                                        in_=st2s[ob], axis=mybir.AxisListType.X,
                                        op=ALU.add)
                nc.vector.tensor_reduce(out=pk2[:, 2 * ob + 1:2 * ob + 2],
                                        in_=st2q[ob], axis=mybir.AxisListType.X,
                                        op=ALU.add)
            cc2i = p_dram.tile([128, 2 * CB], F32, name="cc2i")
            cc2o = p_dram.tile([128, 2 * CB], F32, name="cc2o")
            nc.sync.dma_start(out=cc2i, in_=pk2)
            nc.gpsimd.collective_compute(
                "AllReduce", ALU.add, replica_groups=groups,
                ins=[cc2i.opt()], outs=[cc2o.opt()])
            red2 = p_const.tile([128, 2 * CB], F32, name="red2")
            nc.sync.dma_start(out=red2, in_=cc2o)
            q3 = red2.rearrange("p (b k) -> p b k", k=2)

            m2 = p_const.tile([128, CB], F32, name="m2")
            nc.vector.tensor_scalar(out=m2, in0=q3[:, :, 0], scalar1=1.0 / NT,
                                    scalar2=None, op0=ALU.mult)
            e2 = p_const.tile([128, CB], F32, name="e2")
            nc.vector.tensor_scalar(out=e2, in0=q3[:, :, 1], scalar1=1.0 / NT,
                                    scalar2=None, op0=ALU.mult)
            mm2 = p_const.tile([128, CB], F32, name="mm2")
            nc.vector.tensor_mul(mm2, m2, m2)
            v2 = p_const.tile([128, CB], F32, name="v2")
            nc.vector.tensor_sub(v2, e2, mm2)
            v2f = p_const.tile([128, CB], F32, name="v2f")
            nc.vector.tensor_scalar(out=v2f, in0=v2, scalar1=EPS, scalar2=None,
                                    op0=ALU.add)
            rc2 = p_const.tile([128, CB], F32, name="rc2")
            nc.vector.reciprocal(rc2, v2f)
            rstd2 = p_const.tile([128, CB], F32, name="rstd2")
            nc.scalar.activation(rstd2, rc2, AF.Sqrt)
            fscale = p_const.tile([128, CB], F32, name="fscale")
            nc.vector.tensor_mul(fscale, g2t, rstd2)
            msc = p_const.tile([128, CB], F32, name="msc")
            nc.vector.tensor_mul(msc, m2, fscale)
            fbias = p_const.tile([128, CB], F32, name="fbias")
            nc.vector.tensor_sub(fbias, b2t, msc)

            # ---- final: clip(z * fscale + fbias) -> DRAM ----
            # affine split across ScalarE (ob=0) / VectorE (ob=1); clamp on DVE
            for n in range(IMGS):
                for ob in range(CB):
                    o1 = p_o1.tile([128, PIX], F32, tag="o1")
                    if ob == 0:
                        nc.scalar.activation(o1, z[n][ob], AF.Identity,
                                             bias=fbias[:, ob:ob + 1],
                                             scale=fscale[:, ob:ob + 1])
                    else:
                        nc.vector.tensor_scalar(
                            out=o1, in0=z[n][ob],
                            scalar1=fscale[:, ob:ob + 1],
                            scalar2=fbias[:, ob:ob + 1],
                            op0=ALU.mult, op1=ALU.add)
                    nc.vector.tensor_scalar(out=o1, in0=o1, scalar1=-1.0,
                                            scalar2=1.0, op0=ALU.max,
                                            op1=ALU.min)
                    dma_chunked(
                        out_d[n, ob * 128:(ob + 1) * 128].rearrange(
                            "c h w -> c (h w)"),
                        o1, parts=2)

    nc.compile()
    return nc


def _get_program():
    global _PROGRAM
    if _PROGRAM is None:
        _PROGRAM = _build_program()
    return _PROGRAM


def run_sharded(inputs, **spmd_kwargs):
    """Shard inputs across 8 cores, run, and gather. Returns (out, results)."""
    nc = _get_program()
    x = np.ascontiguousarray(np.asarray(inputs["x"], dtype=np.float32))
    base = {
        k: np.ascontiguousarray(np.asarray(inputs[k], dtype=np.float32))
        for k in ("w1", "w2", "gamma1", "beta1", "gamma2", "beta2")
    }
    shards = np.split(x, N_CORES, axis=0)
    in_maps = [{"x": shards[i], **base} for i in range(N_CORES)]
    res = run_bass_kernel_spmd(nc, in_maps, core_ids=list(range(N_CORES)),
                               **spmd_kwargs)
    out = np.concatenate([res.results[i]["out"] for i in range(N_CORES)],
                         axis=0).astype(np.float32)
    return out, res


def kernel(**inputs):
    out, _ = run_sharded(inputs)
    return out
